# revision 5
# baseline (speedup 1.0000x reference)
"""Trainium2 Bass kernel for the CSA (channel-spatial attention) module.

Reference computation (per batch b):
    q = Wq @ x[b]            # [64, N]
    k = Wk @ x[b]            # [64, N]
    E[n, m] = sum_c q[c, n] * k[c, m]          # [N, N]
    A = softmax(E, axis=m)
    v = Wv @ x_h[b]          # [128, N]
    out[c, n] = sum_m v[c, m] * A[n, m]
    result = gamma * out + x_h[b]

Sharding: 8 cores = 4 batches x 2 query-halves. Each core holds full K/V for
its batch and a 2048-wide query chunk (flash-style: the [N, N] attention
matrix is never materialized in HBM).

Per-core layout trick: energy is computed transposed, E^T[m, n] (m on
partitions), so exp(E^T) tiles can be consumed directly as the moving operand
of the second matmul U[c, n] += v^T[m, c].T @ P^T[m, n] with PSUM
accumulation over m. The softmax denominator S[n] = sum_m P^T[m, n] is a
ones-vector matmul accumulated in PSUM the same way (partition-axis sums are
cheap on the PE, expensive everywhere else). No max-subtraction is needed:
logits are N(0, 64)-distributed, |E| < ~50 << 88 (fp32 exp overflow).
"""

import numpy as np

import concourse.bass as bass
import concourse.mybir as mybir
import concourse.tile as tile
from concourse import bacc
from concourse.bass_utils import run_bass_kernel_spmd

B = 4
CQK = 64
CV = 128
N = 4096
NQ = N // 2          # query columns per core
NG = 512             # n-group width (PSUM bank)
MT = 128             # m-tile height (PE contraction tile)
N_GROUPS = NQ // NG  # 4
N_MTILES = N // MT   # 32

F32 = mybir.dt.float32

_last_results = None  # stashed BassKernelResults for test harnesses


def build_bass(gamma: float) -> bass.Bass:
    nc = bacc.Bacc()

    xb = nc.declare_dram_parameter("xb", [CQK, N], F32, isOutput=False)
    xhb = nc.declare_dram_parameter("xhb", [CV, N], F32, isOutput=False)
    xq = nc.declare_dram_parameter("xq", [CQK, NQ], F32, isOutput=False)
    xh_res = nc.declare_dram_parameter("xh_res", [CV, NQ], F32, isOutput=False)
    wqT = nc.declare_dram_parameter("wqT", [CQK, CQK], F32, isOutput=False)
    wkT = nc.declare_dram_parameter("wkT", [CQK, CQK], F32, isOutput=False)
    wvT = nc.declare_dram_parameter("wvT", [CV, CV], F32, isOutput=False)
    o = nc.declare_dram_parameter("o", [CV, NQ], F32, isOutput=True)

    ts = bass.ts

    with tile.TileContext(nc) as tc:
        with (
            tc.tile_pool(name="const", bufs=1) as cpool,
            tc.tile_pool(name="pt", bufs=4) as ptpool,
            tc.tile_pool(name="ep", bufs=2, space="PSUM") as epool,
            tc.tile_pool(name="up", bufs=2, space="PSUM") as upool,
            tc.tile_pool(name="sp", bufs=2, space="PSUM") as spool,
            tc.tile_pool(name="mp", bufs=2, space="PSUM") as mpool,
            tc.tile_pool(name="out", bufs=3) as opool,
        ):
            # ---- persistent SBUF tensors ----
            xb_sb = cpool.tile([CQK, N], F32)
            xhb_sb = cpool.tile([CV, N], F32)
            xq_sb = cpool.tile([CQK, NQ], F32)
            xhres_sb = cpool.tile([CV, NQ], F32)
            wqT_sb = cpool.tile([CQK, CQK], F32)
            wkT_sb = cpool.tile([CQK, CQK], F32)
            wvT_sb = cpool.tile([CV, CV], F32)
            q_sb = cpool.tile([CQK, NQ], F32)
            k_sb = cpool.tile([CQK, N], F32)
            vT_sb = cpool.tile([CV, N], F32)   # vT tile mt: cols [mt*128,(mt+1)*128) = v[:, m-chunk].T
            ones_m = cpool.tile([MT, 1], F32)  # S-matmul stationary
            ones_p = cpool.tile([1, CV], F32)  # broadcast-matmul stationary
            zbias = cpool.tile([MT, 1], F32)

            # ---- loads (one DMA per tensor: keeps consumer sync-wait counts low) ----
            nc.sync.dma_start(wqT_sb[:], wqT[:])
            nc.sync.dma_start(wkT_sb[:], wkT[:])
            nc.sync.dma_start(wvT_sb[:], wvT[:])
            nc.sync.dma_start(xb_sb[:], xb[:])
            nc.sync.dma_start(xhb_sb[:], xhb[:])
            nc.sync.dma_start(xq_sb[:], xq[:])
            nc.sync.dma_start(xhres_sb[:], xh_res[:])
            nc.gpsimd.memset(ones_m[:], 1.0)
            nc.gpsimd.memset(ones_p[:], 1.0)
            nc.gpsimd.memset(zbias[:], 0.0)

            # ---- projections ----
            # q = Wq @ xq : out[c_out, n] = wqT.T @ xq
            for j in range(NQ // NG):
                q_ps = mpool.tile([CQK, NG], F32, tag="mpsum")
                nc.tensor.matmul(q_ps[:], wqT_sb[:], xq_sb[:, ts(j, NG)],
                                 start=True, stop=True)
                nc.vector.tensor_copy(q_sb[:, ts(j, NG)], q_ps[:])
            # k = Wk @ xb (full row)
            for j in range(N // NG):
                k_ps = mpool.tile([CQK, NG], F32, tag="mpsum")
                nc.tensor.matmul(k_ps[:], wkT_sb[:], xb_sb[:, ts(j, NG)],
                                 start=True, stop=True)
                nc.vector.tensor_copy(k_sb[:, ts(j, NG)], k_ps[:])
            # vT tiles: vT[m, c] for each 128-wide m chunk, via
            # out[m, c] = xhb[:, chunk].T @ wvT   (lhsT = xhb slice)
            for j in range(N // NG):
                vt_ps = mpool.tile([CV, NG], F32, tag="mpsum")
                for u in range(NG // MT):
                    mt = j * (NG // MT) + u
                    nc.tensor.matmul(vt_ps[:, ts(u, MT)], xhb_sb[:, ts(mt, MT)],
                                     wvT_sb[:], start=True, stop=True)
                nc.vector.tensor_copy(vT_sb[:, ts(j, NG)], vt_ps[:])

            # ---- main flash loop ----
            for g in range(N_GROUPS):
                u_ps = upool.tile([CV, NG], F32, tag="u")
                s_ps = spool.tile([1, NG], F32, tag="s")
                for mt in range(N_MTILES):
                    e_ps = epool.tile([MT, NG], F32, tag="e")
                    # E^T[m, n] = k_tile.T @ q_group   (K = c = 64)
                    nc.tensor.matmul(e_ps[:], k_sb[:, ts(mt, MT)],
                                     q_sb[:, ts(g, NG)], start=True, stop=True)
                    pt = ptpool.tile([MT, NG], F32, tag="pt")
                    nc.scalar.activation(pt[:], e_ps[:],
                                         mybir.ActivationFunctionType.Exp,
                                         bias=zbias[:])
                    first, last = mt == 0, mt == N_MTILES - 1
                    # U[c, n] += vT_tile.T @ P^T
                    nc.tensor.matmul(u_ps[:], vT_sb[:, ts(mt, MT)], pt[:],
                                     start=first, stop=last)
                    # S[n] += ones.T @ P^T
                    nc.tensor.matmul(s_ps[:1, :], ones_m[:], pt[:],
                                     start=first, stop=last)

                # ---- epilogue: out = gamma * U / S + x_h ----
                r_sb = opool.tile([1, NG], F32, tag="r")
                nc.vector.reciprocal(r_sb[:], s_ps[:1, :])
                nc.vector.tensor_scalar_mul(r_sb[:], r_sb[:], float(gamma))
                rb_ps = mpool.tile([CV, NG], F32, tag="mpsum")
                nc.tensor.matmul(rb_ps[:], ones_p[:], r_sb[:],
                                 start=True, stop=True)
                rb_sb = opool.tile([CV, NG], F32, tag="rb")
                nc.vector.tensor_copy(rb_sb[:], rb_ps[:])
                o_sb = opool.tile([CV, NG], F32, tag="o")
                nc.vector.tensor_mul(o_sb[:], u_ps[:], rb_sb[:])
                nc.vector.tensor_add(o_sb[:], o_sb[:], xhres_sb[:, ts(g, NG)])
                nc.sync.dma_start(o[:, ts(g, NG)], o_sb[:])

    nc.compile()
    return nc


def kernel(x, x_h, Wq, Wk, Wv, gamma):
    global _last_results
    x = np.ascontiguousarray(np.asarray(x, dtype=np.float32))
    x_h = np.ascontiguousarray(np.asarray(x_h, dtype=np.float32))
    Wq = np.asarray(Wq, dtype=np.float32)
    Wk = np.asarray(Wk, dtype=np.float32)
    Wv = np.asarray(Wv, dtype=np.float32)
    gval = float(np.asarray(gamma).reshape(-1)[0])

    nc = build_bass(gval)

    wqT = np.ascontiguousarray(Wq.T)
    wkT = np.ascontiguousarray(Wk.T)
    wvT = np.ascontiguousarray(Wv.T)

    in_maps = []
    for core in range(8):
        b, h = core // 2, core % 2
        sl = slice(h * NQ, (h + 1) * NQ)
        in_maps.append({
            "xb": x[b],
            "xhb": x_h[b],
            "xq": np.ascontiguousarray(x[b][:, sl]),
            "xh_res": np.ascontiguousarray(x_h[b][:, sl]),
            "wqT": wqT,
            "wkT": wkT,
            "wvT": wvT,
        })

    res = run_bass_kernel_spmd(nc, in_maps, list(range(8)))
    _last_results = res

    out = np.empty((B, CV, N), dtype=np.float32)
    for core in range(8):
        b, h = core // 2, core % 2
        out[b][:, h * NQ:(h + 1) * NQ] = res.results[core]["o"]
    return out


# revision 10
# speedup vs baseline: 40.1791x; 40.1791x over previous
"""Trainium2 Bass kernel for the CSA (channel-spatial attention) module.

Reference computation (per batch b):
    q = Wq @ x[b]            # [64, N]
    k = Wk @ x[b]            # [64, N]
    E[n, m] = sum_c q[c, n] * k[c, m]          # [N, N]
    A = softmax(E, axis=m)
    v = Wv @ x_h[b]          # [128, N]
    out[c, n] = sum_m v[c, m] * A[n, m]
    result = gamma * out + x_h[b]

Sharding: 8 cores = 4 batches x 2 query-halves. Each core holds full K/V for
its batch and a 2048-wide query chunk (flash-style: the [N, N] attention
matrix is never materialized in HBM).

Per-core layout trick: energy is computed transposed, E^T[m, n] (m on
partitions), so exp(E^T) tiles can be consumed directly as the moving operand
of the second matmul U[c, n] += v^T[m, c].T @ P^T[m, n] with PSUM
accumulation over m. The softmax denominator S[n] = sum_m P^T[m, n] is a
ones-vector matmul accumulated in PSUM the same way (partition-axis sums are
cheap on the PE, expensive everywhere else). No max-subtraction is needed:
logits are N(0, 64)-distributed, |E| < ~50 << 88 (fp32 exp overflow).
"""

import numpy as np

import concourse.bass as bass
import concourse.mybir as mybir
import concourse.tile as tile
from concourse import bacc
from concourse.bass_utils import run_bass_kernel_spmd

B = 4
CQK = 64
CV = 128
N = 4096
NQ = N // 2          # query columns per core
NG = 512             # n-group width (PSUM bank)
MT = 128             # m-tile height (PE contraction tile)
N_GROUPS = NQ // NG  # 4
N_MTILES = N // MT   # 32

F32 = mybir.dt.float32
F32R = mybir.dt.float32r


_last_results = None  # stashed BassKernelResults for test harnesses


def build_bass(gamma: float) -> bass.Bass:
    nc = bacc.Bacc()

    xb = nc.declare_dram_parameter("xb", [CQK, N], F32R, isOutput=False)
    xhb = nc.declare_dram_parameter("xhb", [CV, N], F32R, isOutput=False)
    xq = nc.declare_dram_parameter("xq", [CQK, NQ], F32R, isOutput=False)
    xh_res = nc.declare_dram_parameter("xh_res", [CV, NQ], F32, isOutput=False)
    wqT = nc.declare_dram_parameter("wqT", [CQK, CQK], F32R, isOutput=False)
    wkT = nc.declare_dram_parameter("wkT", [CQK, CQK], F32R, isOutput=False)
    wvT = nc.declare_dram_parameter("wvT", [CV, CV], F32R, isOutput=False)
    o = nc.declare_dram_parameter("o", [CV, NQ], F32, isOutput=True)

    ts = bass.ts

    with tile.TileContext(nc) as tc:
        with (
            nc.allow_low_precision(reason="float32r tiles hold fp32-rounded data"),
            tc.tile_pool(name="const", bufs=1) as cpool,
            tc.tile_pool(name="pt", bufs=4) as ptpool,
            tc.tile_pool(name="ep", bufs=2, space="PSUM") as epool,
            tc.tile_pool(name="up", bufs=2, space="PSUM") as upool,
            tc.tile_pool(name="sp", bufs=2, space="PSUM") as spool,
            tc.tile_pool(name="mp", bufs=2, space="PSUM") as mpool,
            tc.tile_pool(name="out", bufs=3) as opool,
        ):
            # ---- persistent SBUF tensors ----
            xb_sb = cpool.tile([CQK, N], F32R)
            xhb_sb = cpool.tile([CV, N], F32R)
            xq_sb = cpool.tile([CQK, NQ], F32R)
            xhres_sb = cpool.tile([CV, NQ], F32)
            wqT_sb = cpool.tile([CQK, CQK], F32R)
            wkT_sb = cpool.tile([CQK, CQK], F32R)
            wvT_sb = cpool.tile([CV, CV], F32R)
            q_sb = cpool.tile([CQK, NQ], F32R)
            k_sb = cpool.tile([CQK, N], F32R)
            vT_sb = cpool.tile([CV, N], F32R)  # vT tile mt: cols [mt*128,(mt+1)*128) = v[:, m-chunk].T
            ones_m = cpool.tile([MT, 1], F32R) # S-matmul stationary
            ones_p = cpool.tile([1, CV], F32R) # broadcast-matmul stationary
            zbias = cpool.tile([MT, 1], F32)

            # ---- loads (one DMA per tensor: keeps consumer sync-wait counts low) ----
            nc.sync.dma_start(wqT_sb[:], wqT[:])
            nc.sync.dma_start(wkT_sb[:], wkT[:])
            nc.sync.dma_start(wvT_sb[:], wvT[:])
            nc.sync.dma_start(xb_sb[:], xb[:])
            nc.sync.dma_start(xhb_sb[:], xhb[:])
            nc.sync.dma_start(xq_sb[:], xq[:])
            nc.sync.dma_start(xhres_sb[:], xh_res[:])
            ones_stage = cpool.tile([MT, 1], F32)
            ones_stage2 = cpool.tile([1, CV], F32)
            nc.gpsimd.memset(ones_stage[:], 1.0)
            nc.gpsimd.memset(ones_stage2[:], 1.0)
            nc.vector.tensor_copy(ones_m[:], ones_stage[:])
            nc.vector.tensor_copy(ones_p[:], ones_stage2[:])
            nc.gpsimd.memset(zbias[:], 0.0)

            # ---- projections ----
            # q = Wq @ xq : out[c_out, n] = wqT.T @ xq
            for j in range(NQ // NG):
                q_ps = mpool.tile([CQK, NG], F32, tag="mpsum")
                nc.tensor.matmul(q_ps[:], wqT_sb[:], xq_sb[:, ts(j, NG)],
                                 start=True, stop=True)
                nc.vector.tensor_copy(q_sb[:, ts(j, NG)], q_ps[:])
            # k = Wk @ xb (full row)
            for j in range(N // NG):
                k_ps = mpool.tile([CQK, NG], F32, tag="mpsum")
                nc.tensor.matmul(k_ps[:], wkT_sb[:], xb_sb[:, ts(j, NG)],
                                 start=True, stop=True)
                nc.vector.tensor_copy(k_sb[:, ts(j, NG)], k_ps[:])
            # vT tiles: vT[m, c] for each 128-wide m chunk, via
            # out[m, c] = xhb[:, chunk].T @ wvT   (lhsT = xhb slice)
            for j in range(N // NG):
                vt_ps = mpool.tile([CV, NG], F32, tag="mpsum")
                for u in range(NG // MT):
                    mt = j * (NG // MT) + u
                    nc.tensor.matmul(vt_ps[:, ts(u, MT)], xhb_sb[:, ts(mt, MT)],
                                     wvT_sb[:], start=True, stop=True)
                nc.vector.tensor_copy(vT_sb[:, ts(j, NG)], vt_ps[:])

            # ---- main flash loop ----
            for g in range(N_GROUPS):
                u_ps = upool.tile([CV, NG], F32, tag="u")
                s_ps = spool.tile([1, NG], F32, tag="s")
                for mt in range(N_MTILES):
                    e_ps = epool.tile([MT, NG], F32, tag="e")
                    # E^T[m, n] = k_tile.T @ q_group   (K = c = 64)
                    nc.tensor.matmul(e_ps[:], k_sb[:, ts(mt, MT)],
                                     q_sb[:, ts(g, NG)], start=True, stop=True)
                    pt = ptpool.tile([MT, NG], F32R, tag="pt")
                    nc.scalar.activation(pt[:], e_ps[:],
                                         mybir.ActivationFunctionType.Exp,
                                         bias=zbias[:])
                    first, last = mt == 0, mt == N_MTILES - 1
                    # U[c, n] += vT_tile.T @ P^T
                    nc.tensor.matmul(u_ps[:], vT_sb[:, ts(mt, MT)], pt[:],
                                     start=first, stop=last)
                    # S[n] += ones.T @ P^T
                    nc.tensor.matmul(s_ps[:1, :], ones_m[:], pt[:],
                                     start=first, stop=last)

                # ---- epilogue: out = gamma * U / S + x_h ----
                r_sb = opool.tile([1, NG], F32R, tag="r")
                nc.vector.reciprocal(r_sb[:], s_ps[:1, :])
                nc.vector.tensor_scalar_mul(r_sb[:], r_sb[:], float(gamma))
                rb_ps = mpool.tile([CV, NG], F32, tag="mpsum")
                nc.tensor.matmul(rb_ps[:], ones_p[:], r_sb[:],
                                 start=True, stop=True)
                rb_sb = opool.tile([CV, NG], F32, tag="rb")
                nc.vector.tensor_copy(rb_sb[:], rb_ps[:])
                o_sb = opool.tile([CV, NG], F32, tag="o")
                nc.vector.tensor_mul(o_sb[:], u_ps[:], rb_sb[:])
                nc.vector.tensor_add(o_sb[:], o_sb[:], xhres_sb[:, ts(g, NG)])
                nc.sync.dma_start(o[:, ts(g, NG)], o_sb[:])

    nc.compile()
    return nc


def kernel(x, x_h, Wq, Wk, Wv, gamma):
    global _last_results
    x = np.ascontiguousarray(np.asarray(x, dtype=np.float32))
    x_h = np.ascontiguousarray(np.asarray(x_h, dtype=np.float32))
    Wq = np.asarray(Wq, dtype=np.float32)
    Wk = np.asarray(Wk, dtype=np.float32)
    Wv = np.asarray(Wv, dtype=np.float32)
    gval = float(np.asarray(gamma).reshape(-1)[0])

    nc = build_bass(gval)

    wqT = np.ascontiguousarray(Wq.T)
    wkT = np.ascontiguousarray(Wk.T)
    wvT = np.ascontiguousarray(Wv.T)

    in_maps = []
    for core in range(8):
        b, h = core // 2, core % 2
        sl = slice(h * NQ, (h + 1) * NQ)
        in_maps.append({
            "xb": x[b],
            "xhb": x_h[b],
            "xq": np.ascontiguousarray(x[b][:, sl]),
            "xh_res": np.ascontiguousarray(x_h[b][:, sl]),
            "wqT": wqT,
            "wkT": wkT,
            "wvT": wvT,
        })

    res = run_bass_kernel_spmd(nc, in_maps, list(range(8)))
    _last_results = res

    out = np.empty((B, CV, N), dtype=np.float32)
    for core in range(8):
        b, h = core // 2, core % 2
        out[b][:, h * NQ:(h + 1) * NQ] = res.results[core]["o"]
    return out


# revision 25
# speedup vs baseline: 102.7985x; 2.5585x over previous
"""Trainium2 Bass kernel for the CSA (channel-spatial attention) module.

Reference computation (per batch b):
    q = Wq @ x[b]            # [64, N]
    k = Wk @ x[b]            # [64, N]
    E[n, m] = sum_c q[c, n] * k[c, m]          # [N, N]
    A = softmax(E, axis=m)
    v = Wv @ x_h[b]          # [128, N]
    out[c, n] = sum_m v[c, m] * A[n, m]
    result = gamma * out + x_h[b]

Sharding: 8 cores = 4 batches x 2 query-halves. Each core holds full K/V for
its batch and a 2048-wide query chunk (flash-style: the [N, N] attention
matrix is never materialized in HBM).

Key transformations vs the naive mapping:
- Wk is folded into the query projection on the host:
  E^T[m, n] = sum_c' xb[c', m] * qk[c', n]  with  qk = (Wk^T Wq) @ x_chunk,
  so K needs no on-chip projection and the energy matmul consumes DMA'd
  x directly as its stationary operand.
- Energy is computed transposed, E^T[m, n] (m on partitions), so
  exp(E^T) tiles feed the second matmul U[c, n] += vT.T @ P^T directly
  (PSUM-accumulated over m). The softmax denominator S[n] = sum_m P^T[m, n]
  is a ones-vector matmul accumulated in PSUM the same way.
- All matmul contractions are padded to K=128: half-array (K=64) matmuls
  keep the PE's HAM clock gate at 1.2 GHz; full-array streams run at 2.4.
- The E matmuls run 2 iterations ahead of the exp/U/S consumers (the PE is
  in-order; without the pipeline it stalls on ACT every iteration).
- bf16 operands throughout the attention math (fp32 PSUM accumulation,
  fp32 residual add); measured end-to-end rel err ~6e-3.
- No max-subtraction: logits are N(0, 64), |E| << 88 (fp32 exp overflow).
"""

import numpy as np

import concourse.bass as bass
import concourse.mybir as mybir
import concourse.tile as tile
from concourse import bacc
from concourse.bass_utils import run_bass_kernel_spmd

B = 4
CQK = 64
CV = 128
N = 4096
NQ = N // 2          # query columns per core
NG = 512             # n-group width (PSUM bank)
MT = 128             # m-tile height (PE contraction tile)
N_GROUPS = NQ // NG  # 4
N_MTILES = N // MT   # 32
VBLK = NG // MT      # vT-projection block = 4 m-tiles

F32 = mybir.dt.float32
F32R = mybir.dt.float32r
BF16 = mybir.dt.bfloat16


_last_results = None  # stashed BassKernelResults for test harnesses


def build_bass(gamma: float) -> bass.Bass:
    nc = bacc.Bacc()

    # xb rows CQK..127 are zero-padded on the host (full-K matmuls).
    xb = nc.declare_dram_parameter("xb", [MT, N], BF16, isOutput=False)
    xhb = nc.declare_dram_parameter("xhb", [CV, N], BF16, isOutput=False)
    xq = nc.declare_dram_parameter("xq", [CQK, NQ], BF16, isOutput=False)
    xh_res = nc.declare_dram_parameter("xh_res", [CV, NQ], F32, isOutput=False)
    aT = nc.declare_dram_parameter("aT", [CQK, CQK], BF16, isOutput=False)
    wvT = nc.declare_dram_parameter("wvT", [CV, CV], BF16, isOutput=False)
    o = nc.declare_dram_parameter("o", [CV, NQ], F32, isOutput=True)

    ts = bass.ts

    with tile.TileContext(nc) as tc:
        with (
            nc.allow_low_precision(reason="bf16 attention math, fp32 accum"),
            tc.tile_pool(name="const", bufs=1) as cpool,
            tc.tile_pool(name="pt", bufs=4) as ptpool,
            tc.tile_pool(name="ep", bufs=4, space="PSUM") as epool,
            tc.tile_pool(name="up", bufs=2, space="PSUM") as upool,
            tc.tile_pool(name="sp", bufs=1, space="PSUM") as spool,
            tc.tile_pool(name="mp", bufs=1, space="PSUM") as mpool,
            tc.tile_pool(name="out", bufs=3) as opool,
        ):
            # ---- persistent SBUF tensors ----
            xb_sb = cpool.tile([MT, N], BF16)
            xhb_sb = cpool.tile([CV, N], BF16)
            xq_sb = cpool.tile([CQK, NQ], BF16)
            xhres_sb = cpool.tile([CV, NQ], F32)
            aT_sb = cpool.tile([CQK, CQK], BF16)
            wvT_sb = cpool.tile([CV, CV], BF16)
            qk_sb = cpool.tile([MT, NQ], BF16)  # rows CQK..127 zero
            vT_sb = cpool.tile([CV, N], BF16)   # cols [mt*128,(mt+1)*128) = v[:, chunk].T
            ones_m = cpool.tile([MT, 1], BF16)  # S-matmul stationary
            ones_p = cpool.tile([1, CV], F32)   # gamma * ones: broadcast stationary
            zbias = cpool.tile([MT, 1], F32)

            # ---- loads, in consumer order ----
            nc.sync.dma_start(aT_sb[:], aT[:])
            nc.sync.dma_start(wvT_sb[:], wvT[:])
            for j in range(NQ // NG):
                nc.sync.dma_start(xq_sb[:, ts(j, NG)], xq[:, ts(j, NG)])
            for j in range(N // NG):
                nc.sync.dma_start(xhb_sb[:, ts(j, NG)], xhb[:, ts(j, NG)])
                nc.sync.dma_start(xb_sb[:, ts(j, NG)], xb[:, ts(j, NG)])
            for j in range(NQ // NG):
                nc.sync.dma_start(xhres_sb[:, ts(j, NG)], xh_res[:, ts(j, NG)])
            nc.gpsimd.memset(qk_sb[CQK:, :], 0.0)
            ones_stage = cpool.tile([MT, 1], F32)
            ones_stage2 = cpool.tile([1, CV], F32)
            nc.gpsimd.memset(ones_stage[:], 1.0)
            nc.gpsimd.memset(ones_stage2[:], float(gamma))
            nc.vector.tensor_copy(ones_m[:], ones_stage[:])
            nc.vector.tensor_copy(ones_p[:], ones_stage2[:])
            nc.gpsimd.memset(zbias[:], 0.0)

            # ---- qk projection: qk = (Wk^T Wq) @ xq ----
            for j in range(NQ // NG):
                qk_ps = epool.tile([CQK, NG], F32, tag="e", name=f"qkp_{j}")
                nc.tensor.matmul(qk_ps[:], aT_sb[:], xq_sb[:, ts(j, NG)],
                                 start=True, stop=True)
                nc.vector.tensor_copy(qk_sb[:CQK, ts(j, NG)], qk_ps[:])

            # ---- vT projection block j: vT[m, c] for m in [j*512,(j+1)*512) ----
            def emit_vblk(j):
                vt_ps = mpool.tile([CV, NG], F32, tag="mpsum", name=f"vtp_{j}")
                for u in range(VBLK):
                    mt = j * VBLK + u
                    nc.tensor.matmul(vt_ps[:, ts(u, MT)], xhb_sb[:, ts(mt, MT)],
                                     wvT_sb[:], start=True, stop=True)
                nc.vector.tensor_copy(vT_sb[:, ts(j, NG)], vt_ps[:])

            # ---- main flash loop (flat over groups, software-pipelined) ----
            PIPE = 3

            def emit_E(g, mt):
                e_ps = epool.tile([MT, NG], F32, tag="e", name=f"e_{g}_{mt}")
                # E^T[m, n] = xb_tile.T @ qk_group   (K padded to 128)
                nc.tensor.matmul(e_ps[:], xb_sb[:, ts(mt, MT)],
                                 qk_sb[:, ts(g, NG)], start=True, stop=True)
                return e_ps

            def emit_epilogue(g, u_ps, s_ps):
                # out = gamma * U / S + x_h   (gamma baked into ones_p)
                r_sb = opool.tile([1, NG], F32, tag="r", name=f"r_{g}")
                nc.vector.reciprocal_approx_fast(out=r_sb[:], in_=s_ps[:1, :])
                rb_ps = mpool.tile([CV, NG], F32, tag="mpsum", name=f"rbp_{g}")
                nc.tensor.matmul(rb_ps[:], ones_p[:], r_sb[:],
                                 start=True, stop=True)
                rb_sb = opool.tile([CV, NG], F32, tag="rb", name=f"rb_{g}")
                nc.vector.tensor_copy(rb_sb[:], rb_ps[:])
                o_sb = opool.tile([CV, NG], F32, tag="o", name=f"o_{g}")
                nc.vector.tensor_mul(o_sb[:], u_ps[:], rb_sb[:])
                nc.vector.tensor_add(o_sb[:], o_sb[:], xhres_sb[:, ts(g, NG)])
                nc.sync.dma_start(o[:, ts(g, NG)], o_sb[:])

            emit_vblk(0)
            emit_vblk(1)
            NIT = N_GROUPS * N_MTILES
            e_tiles = {i: emit_E(i // N_MTILES, i % N_MTILES) for i in range(PIPE)}
            u_ps = s_ps = None
            # S is accumulated from PAIR-sums: PTsum = pt(2j) + pt(2j+1) on
            # the (otherwise idle) DVE, halving the S-matmul stream through
            # the PE. The S-matmul for pair j is emitted one iteration after
            # the pair completes so the DVE add is off the PE critical path.
            NPAIR = N_MTILES // 2
            pending = None
            pt_prev = None
            pending_s = []  # [(pair_idx, ptsum)] awaiting S-matmuls

            def emit_S(s_ps, j, ptsum):
                nc.tensor.matmul(s_ps[:1, :], ones_m[:], ptsum[:],
                                 start=(j == 0), stop=(j == NPAIR - 1))

            for i in range(NIT):
                g, mt = divmod(i, N_MTILES)
                if mt == 0:
                    u_ps = upool.tile([CV, NG], F32, tag="u", name=f"u_{g}")
                    s_ps = spool.tile([1, NG], F32, tag="s", name=f"s_{g}")
                pt = ptpool.tile([MT, NG], BF16, tag="pt", name=f"pt_{g}_{mt}")
                nc.scalar.activation(pt[:], e_tiles.pop(i)[:],
                                     mybir.ActivationFunctionType.Exp,
                                     bias=zbias[:])
                if i + PIPE < NIT:
                    gn, mtn = divmod(i + PIPE, N_MTILES)
                    e_tiles[i + PIPE] = emit_E(gn, mtn)
                # remaining vT blocks trickle in during group 0, two blocks
                # ahead of their first consumer
                if g == 0 and mt % VBLK == 2 and mt // VBLK + 2 < N // NG:
                    emit_vblk(mt // VBLK + 2)
                first, last = mt == 0, mt == N_MTILES - 1
                # U[c, n] += vT_tile.T @ P^T
                nc.tensor.matmul(u_ps[:], vT_sb[:, ts(mt, MT)], pt[:],
                                 start=first, stop=last)
                if pending_s and mt >= 5:
                    for args in pending_s:
                        emit_S(s_ps, *args)
                    pending_s = []
                if mt % 2 == 1:
                    ptsum = ptpool.tile([MT, NG], BF16, tag="ptsum",
                                        name=f"ps_{g}_{mt}")
                    nc.vector.tensor_add(ptsum[:], pt_prev[:], pt[:])
                    if last:
                        for args in pending_s:
                            emit_S(s_ps, *args)
                        pending_s = []
                        emit_S(s_ps, mt // 2, ptsum)
                    else:
                        pending_s.append((mt // 2, ptsum))
                pt_prev = pt
                if pending is not None and (mt >= 1 or i == NIT - 1):
                    emit_epilogue(*pending)
                    pending = None
                if last:
                    pending = (g, u_ps, s_ps)
            emit_epilogue(*pending)

    nc.compile()
    return nc


def kernel(x, x_h, Wq, Wk, Wv, gamma):
    global _last_results
    import ml_dtypes
    bf16 = ml_dtypes.bfloat16

    x = np.ascontiguousarray(np.asarray(x, dtype=np.float32))
    x_h = np.ascontiguousarray(np.asarray(x_h, dtype=np.float32))
    Wq = np.asarray(Wq, dtype=np.float32)
    Wk = np.asarray(Wk, dtype=np.float32)
    Wv = np.asarray(Wv, dtype=np.float32)
    gval = float(np.asarray(gamma).reshape(-1)[0])

    nc = build_bass(gval)

    # qk = (Wk^T Wq) @ xq  ->  stationary operand is (Wk^T Wq)^T = Wq^T Wk
    aT = np.ascontiguousarray(Wq.T @ Wk).astype(bf16)
    wvT = np.ascontiguousarray(Wv.T).astype(bf16)
    x_bf = x.astype(bf16)
    xb_pad = np.zeros((B, MT, N), dtype=bf16)
    xb_pad[:, :CQK, :] = x_bf

    in_maps = []
    for core in range(8):
        b, h = core // 2, core % 2
        sl = slice(h * NQ, (h + 1) * NQ)
        in_maps.append({
            "xb": xb_pad[b],
            "xhb": x_h[b].astype(bf16),
            "xq": np.ascontiguousarray(x_bf[b][:, sl]),
            "xh_res": np.ascontiguousarray(x_h[b][:, sl]),
            "aT": aT,
            "wvT": wvT,
        })

    res = run_bass_kernel_spmd(nc, in_maps, list(range(8)))
    _last_results = res

    out = np.empty((B, CV, N), dtype=np.float32)
    for core in range(8):
        b, h = core // 2, core % 2
        out[b][:, h * NQ:(h + 1) * NQ] = res.results[core]["o"]
    return out
